# revision 7
# baseline (speedup 1.0000x reference)
# Trainium2 Bass kernel for voxel-routed MoE NeRF MLPs (4096 tiny experts).
#
# Strategy: host computes the voxel index per point, sorts points by model,
# shards the sorted list across 8 cores, and packs per-block (32 points, one
# model) transposed weights with biases folded in as an extra contraction row.
# The device streams the packed weights and runs, per chunk of 32 blocks
# (1024 points): positional-encoding matmuls + Sin activations, then 5 small
# matmuls per block (layer0, layer1, feat+sigma fused, view, rgb).
import numpy as np

RES = 16
L_XYZ = 10
L_DIR = 4
W = 32
N_RAYS, N_SAMPLES = 1024, 128
NM = RES * RES * RES
N_CORES = 8
P = 32          # points per block (one model per block)
BPC = 32        # blocks per chunk -> 1024 points per chunk

TRACE = False
LAST_EXEC_NS = None

_cache = {}


def _build_bass(C):
    import concourse.bass as bass
    import concourse.mybir as mybir
    from concourse.tile import TileContext

    dt = mybir.dt.float32
    Sin = mybir.ActivationFunctionType.Sin
    NPT = BPC * P  # 1024 points per chunk

    nc = bass.Bass()
    w0t_h = nc.dram_tensor("w0t", [C, 64, NPT], dt, kind="ExternalInput")
    w1t_h = nc.dram_tensor("w1t", [C, 33, NPT], dt, kind="ExternalInput")
    wfst_h = nc.dram_tensor("wfst", [C, 33, 33 * BPC], dt, kind="ExternalInput")
    wvt_h = nc.dram_tensor("wvt", [C, 60, NPT], dt, kind="ExternalInput")
    wrgbt_h = nc.dram_tensor("wrgbt", [C, 33, 3 * BPC], dt, kind="ExternalInput")
    pts_h = nc.dram_tensor("pts", [C, 3, NPT], dt, kind="ExternalInput")
    vd_h = nc.dram_tensor("vd", [C, 3, NPT], dt, kind="ExternalInput")
    aep_h = nc.dram_tensor("aep", [3, 60], dt, kind="ExternalInput")
    aev_h = nc.dram_tensor("aev", [3, 24], dt, kind="ExternalInput")
    sb_h = nc.dram_tensor("sb", [128, 2], dt, kind="ExternalInput")
    rgb_o = nc.dram_tensor("rgb_o", [C, 3, NPT], dt, kind="ExternalOutput")
    sig_o = nc.dram_tensor("sig_o", [C, 1, NPT], dt, kind="ExternalOutput")

    with TileContext(nc) as tc:
        with (
            tc.tile_pool(name="const", bufs=1) as cpool,
            tc.tile_pool(name="wts", bufs=2) as wpool,
            tc.tile_pool(name="acts", bufs=2) as apool,
            tc.tile_pool(name="ps", bufs=4, space="PSUM") as pspool,
        ):
            aep = cpool.tile([3, 60], dt)
            nc.sync.dma_start(aep[:], aep_h[:])
            aev = cpool.tile([3, 24], dt)
            nc.sync.dma_start(aev[:], aev_h[:])
            sb = cpool.tile([128, 2], dt)
            nc.sync.dma_start(sb[:], sb_h[:])

            for c in range(C):
                w0t = wpool.tile([64, NPT], dt, tag="w0t")
                nc.sync.dma_start(w0t[:], w0t_h[c])
                w1t = wpool.tile([33, NPT], dt, tag="w1t")
                nc.sync.dma_start(w1t[:], w1t_h[c])
                wfst = wpool.tile([33, 33 * BPC], dt, tag="wfst")
                nc.sync.dma_start(wfst[:], wfst_h[c])
                wvt = wpool.tile([60, NPT], dt, tag="wvt")
                nc.sync.dma_start(wvt[:], wvt_h[c])
                wrgbt = wpool.tile([33, 3 * BPC], dt, tag="wrgbt")
                nc.sync.dma_start(wrgbt[:], wrgbt_h[c])

                ptst = apool.tile([3, NPT], dt, tag="ptst")
                nc.sync.dma_start(ptst[:], pts_h[c])
                vdt = apool.tile([3, NPT], dt, tag="vdt")
                nc.sync.dma_start(vdt[:], vd_h[c])

                # ep rows: [sin(30) | cos(30) | xyz(3) | ones(1)]
                ep = apool.tile([64, NPT], dt, tag="ep")
                nc.sync.dma_start(ep[60:63, :], pts_h[c])
                nc.vector.memset(ep[63:64, :], 1.0)
                pse = pspool.tile([60, NPT], dt, tag="ps")
                nc.tensor.matmul(pse[:], aep[:], ptst[:])
                nc.scalar.activation(ep[0:60, :], pse[:], Sin, bias=sb[0:60, 0:1])

                # h2 rows: [feat(32) | sin(12) | cos(12) | v(3) | ones(1)]
                h2 = apool.tile([60, NPT], dt, tag="h2")
                nc.sync.dma_start(h2[56:59, :], vd_h[c])
                nc.vector.memset(h2[59:60, :], 1.0)
                psv = pspool.tile([56, NPT], dt, tag="ps")
                nc.tensor.matmul(psv[32:56, :], aev[:], vdt[:])
                nc.scalar.activation(h2[32:56, :], psv[32:56, :], Sin,
                                     bias=sb[32:56, 1:2])

                # layer 0
                ps0 = pspool.tile([32, NPT], dt, tag="ps")
                for b in range(BPC):
                    s = slice(b * P, (b + 1) * P)
                    nc.tensor.matmul(ps0[:, s], w0t[:, s], ep[:, s])
                h0 = apool.tile([33, NPT], dt, tag="h0")
                nc.vector.memset(h0[32:33, :], 1.0)
                nc.vector.tensor_scalar_max(h0[0:32, :], ps0[:], 0.0)

                # layer 1
                ps1 = pspool.tile([32, NPT], dt, tag="ps")
                for b in range(BPC):
                    s = slice(b * P, (b + 1) * P)
                    nc.tensor.matmul(ps1[:, s], w1t[:, s], h0[:, s])
                h1 = apool.tile([33, NPT], dt, tag="h1")
                nc.vector.memset(h1[32:33, :], 1.0)
                nc.vector.tensor_scalar_max(h1[0:32, :], ps1[:], 0.0)

                # feat (32 rows) + sigma (1 row), fused
                psfs = pspool.tile([33, NPT], dt, tag="ps")
                for b in range(BPC):
                    s = slice(b * P, (b + 1) * P)
                    nc.tensor.matmul(psfs[:, s], wfst[:, b * 33:(b + 1) * 33],
                                     h1[:, s])
                nc.vector.tensor_copy(h2[0:32, :], psfs[0:32, :])
                sg = apool.tile([33, NPT], dt, tag="sg")
                nc.scalar.copy(sg[32:33, :], psfs[32:33, :])
                nc.sync.dma_start(sig_o[c], sg[32:33, :])

                # view layer
                psvw = pspool.tile([32, NPT], dt, tag="ps")
                for b in range(BPC):
                    s = slice(b * P, (b + 1) * P)
                    nc.tensor.matmul(psvw[:, s], wvt[:, s], h2[:, s])
                h3 = apool.tile([33, NPT], dt, tag="h3")
                nc.vector.memset(h3[32:33, :], 1.0)
                nc.scalar.activation(h3[0:32, :], psvw[:],
                                     mybir.ActivationFunctionType.Relu)

                # rgb layer
                psrgb = pspool.tile([3, NPT], dt, tag="ps")
                for b in range(BPC):
                    s = slice(b * P, (b + 1) * P)
                    nc.tensor.matmul(psrgb[:, s], wrgbt[:, b * 3:(b + 1) * 3],
                                     h3[:, s])
                rg = apool.tile([3, NPT], dt, tag="rg")
                nc.vector.tensor_copy(rg[:], psrgb[:])
                nc.sync.dma_start(rgb_o[c], rg[:])

    return nc


def _embed_mat(L):
    # (3, 3*L) matrix M with M[c, 3k+c] = 2^k ; xf = M.T @ xyz
    m = np.zeros((3, 3 * L), dtype=np.float32)
    for k in range(L):
        for c in range(3):
            m[c, 3 * k + c] = 2.0 ** k
    return m


def _route(pts_flat):
    scaled = np.clip(pts_flat * RES, 0.0, RES - 1.0)
    i3 = scaled.astype(np.int32)
    return i3[:, 0] * (RES * RES) + i3[:, 1] * RES + i3[:, 2]


def _make_blocks(s_idx, s_ord):
    """s_idx: sorted model ids of this shard; s_ord: original point indices.
    Returns (bm [B], src [B,P], valid [B,P])."""
    models, counts = np.unique(s_idx, return_counts=True)
    starts = np.concatenate([[0], np.cumsum(counts)[:-1]]).astype(np.int64)
    nblk = (counts + P - 1) // P
    B = int(nblk.sum())
    bm = np.repeat(models, nblk)
    lb = np.arange(B) - np.repeat(np.cumsum(nblk) - nblk, nblk)
    bstart = np.repeat(starts, nblk) + lb * P
    bn = np.minimum(np.repeat(counts, nblk) - lb * P, P)
    off = np.arange(P)[None, :]
    sl = bstart[:, None] + np.minimum(off, (bn - 1)[:, None])
    src = s_ord[sl]
    valid = off < bn[:, None]
    return bm, src, valid


def kernel(pts, viewdirs, pts_w0, pts_b0, pts_w1, pts_b1, feat_w, feat_b,
           sigma_w, sigma_b, view_w, view_b, rgb_w, rgb_b):
    global LAST_EXEC_NS
    from concourse.bass_utils import run_bass_kernel_spmd

    n_rays, n_samples, _ = pts.shape
    N = n_rays * n_samples
    p = np.ascontiguousarray(pts.reshape(-1, 3), dtype=np.float32)
    idx = _route(p)
    order = np.argsort(idx, kind="stable")
    sidx = idx[order]

    shard = N // N_CORES
    per_core = []
    Bmax = 0
    for core in range(N_CORES):
        lo = core * shard
        bm, src, valid = _make_blocks(sidx[lo:lo + shard], order[lo:lo + shard])
        per_core.append((bm, src, valid))
        Bmax = max(Bmax, len(bm))
    Bmax = ((Bmax + BPC - 1) // BPC) * BPC
    C = Bmax // BPC

    # ---- global augmented transposed weight tables -------------------------
    # reference ep feature order: [xyz(3), then per k: sin3, cos3]
    # our ep row order:           [sin30 (k-major), cos30 (k-major), xyz(3)]
    ks = np.repeat(np.arange(L_XYZ), 3)
    cs = np.tile(np.arange(3), L_XYZ)
    perm_ep = np.concatenate([3 + 6 * ks + cs, 6 + 6 * ks + cs, np.arange(3)])
    w0p = pts_w0[:, :, perm_ep]                                   # (NM,32,63)
    w0aT = np.concatenate([w0p, pts_b0[:, :, None]], 2).transpose(0, 2, 1)
    w1aT = np.concatenate([pts_w1.transpose(0, 2, 1),
                           pts_b1[:, None, :]], 1)                # (NM,33,32)
    fsw = np.concatenate([feat_w, sigma_w], 1)                    # (NM,33,32)
    fsb = np.concatenate([feat_b, sigma_b], 1)                    # (NM,33)
    wfsaT = np.concatenate([fsw.transpose(0, 2, 1), fsb[:, None, :]], 1)
    # our h2 rows: [feat(32), sin12 (k-major), cos12 (k-major), v(3)]
    kv = np.repeat(np.arange(L_DIR), 3)
    cv = np.tile(np.arange(3), L_DIR)
    perm_v = np.concatenate([np.arange(32), 35 + 6 * kv + cv, 38 + 6 * kv + cv,
                             np.arange(32, 35)])
    vwp = view_w[:, :, perm_v]                                    # (NM,32,59)
    wvaT = np.concatenate([vwp.transpose(0, 2, 1), view_b[:, None, :]], 1)
    wrgbaT = np.concatenate([rgb_w.transpose(0, 2, 1), rgb_b[:, None, :]], 1)

    vdirs = np.ascontiguousarray(viewdirs, dtype=np.float32)

    def pack_w(tbl, bm, ncols):
        # tbl: (NM, K, ncols) -> (C, K, ncols*BPC)
        g = tbl[bm]                                               # (B,K,ncols)
        K = g.shape[1]
        g = g.reshape(C, BPC, K, ncols).transpose(0, 2, 1, 3)
        return np.ascontiguousarray(g.reshape(C, K, BPC * ncols), np.float32)

    def pack_pts(arr3, src):
        # arr3[src] -> (C, 3, BPC*P)
        g = arr3[src]                                             # (B,P,3)
        g = g.reshape(C, BPC, P, 3).transpose(0, 3, 1, 2)
        return np.ascontiguousarray(g.reshape(C, 3, BPC * P), np.float32)

    aep = _embed_mat(L_XYZ)
    aep = np.concatenate([aep, aep], 1)                           # (3,60)
    aev = _embed_mat(L_DIR)
    aev = np.concatenate([aev, aev], 1)                           # (3,24)
    sb = np.zeros((128, 2), np.float32)
    sb[30:60, 0] = np.pi / 2
    sb[44:56, 1] = np.pi / 2

    in_maps = []
    for core in range(N_CORES):
        bm, src, valid = per_core[core]
        padB = Bmax - len(bm)
        if padB:
            bm = np.concatenate([bm, np.zeros(padB, bm.dtype)])
            src = np.concatenate([src, np.zeros((padB, P), src.dtype)])
            valid = np.concatenate([valid, np.zeros((padB, P), bool)])
            per_core[core] = (bm, src, valid)
        in_maps.append({
            "w0t": pack_w(w0aT, bm, 32),
            "w1t": pack_w(w1aT, bm, 32),
            "wfst": pack_w(wfsaT, bm, 33),
            "wvt": pack_w(wvaT, bm, 32),
            "wrgbt": pack_w(wrgbaT, bm, 3),
            "pts": pack_pts(p, src),
            "vd": pack_pts(vdirs, src // n_samples),
            "aep": aep, "aev": aev, "sb": sb,
        })

    key = C
    if key not in _cache:
        _cache[key] = _build_bass(C)
    nc = _cache[key]

    res = run_bass_kernel_spmd(nc, in_maps, core_ids=list(range(N_CORES)),
                               trace=TRACE)
    LAST_EXEC_NS = res.exec_time_ns

    rgb_full = np.zeros((N, 3), np.float32)
    sig_full = np.zeros((N, 1), np.float32)
    for core in range(N_CORES):
        bm, src, valid = per_core[core]
        r = res.results[core]["rgb_o"]                            # (C,3,BPC*P)
        s = res.results[core]["sig_o"]                            # (C,1,BPC*P)
        r = r.reshape(C, 3, BPC, P).transpose(0, 2, 3, 1).reshape(-1, 3)
        s = s.reshape(C, 1, BPC, P).transpose(0, 2, 3, 1).reshape(-1, 1)
        vm = valid.reshape(-1)
        dst = src.reshape(-1)[vm]
        rgb_full[dst] = r[vm]
        sig_full[dst] = s[vm]

    return (rgb_full.reshape(n_rays, n_samples, 3),
            sig_full.reshape(n_rays, n_samples, 1))
